# revision 7
# baseline (speedup 1.0000x reference)
"""Multi-head causal attention (dense transformer block) on 8 Trainium2 cores.

Sharding: 2-way data parallel over batch x 4-way tensor parallel over heads.
Core c handles batch c//4 and heads 4*(c%4) .. 4*(c%4)+3.

Per-core pipeline (all activation layouts chosen so no on-device transposes
are needed; host pre-transposes x and the weight shards once):
  1. QT/KT [hd, t] and V [t, hd] projections from xT [d, t].
  2. Attention per (head, q-chunk) with scores computed transposed
     (S^T [k, q]), exp without max-subtraction (scores are O(1) so exp is
     safe in fp32), causal masking via affine_select on the diagonal tiles,
     softmax denominators via ones-vector matmuls, AV accumulated as
     out^T [hd, q].
  3. Output projection final[t, e] = sum_c out^T[c, t] * woT[c, e] (partial
     sum over this core's heads).
  4. ReduceScatter over the 4 cores sharing a batch; host concatenates the
     row shards.
"""

import os
import sys

sys.path.insert(0, "/opt/trn_rl_repo")

import numpy as np

N_CORES = 8
B = 2
T = 2048          # sequence length
D = 2048          # model dim
P = 128           # partitions
HD = 128          # head dim
NHG = 4           # head-groups (cores per batch)
HPC = 4           # heads per core
F = HPC * HD      # 512 per-core q/k/v feature width
TC = 512          # token chunk (matmul free dim)
NTC = T // TC     # 4 token chunks
ND = D // P       # 16 d-subtiles
SCALE = float(HD) ** -0.5

_CACHE = {}


def _build(mm_dtype_name: str):
    import concourse.bacc as bacc
    import concourse.mybir as mybir
    import concourse.tile as tile

    dt = mybir.dt
    f32 = dt.float32
    md = getattr(dt, mm_dtype_name)  # dtype of every PE-input tile

    nc = bacc.Bacc(
        "TRN2", target_bir_lowering=False, debug=False, num_devices=N_CORES
    )

    xT = nc.dram_tensor("xT", [D, T], md, kind="ExternalInput")
    wqT = nc.dram_tensor("wqT", [D, F], md, kind="ExternalInput")
    wkT = nc.dram_tensor("wkT", [D, F], md, kind="ExternalInput")
    wvT = nc.dram_tensor("wvT", [D, F], md, kind="ExternalInput")
    woT = nc.dram_tensor("woT", [F, D], md, kind="ExternalInput")
    out = nc.dram_tensor("out", [T // NHG, D], f32, kind="ExternalOutput")

    with nc.allow_low_precision(reason="float32r matmul-input tiles"), \
         tile.TileContext(nc) as tc:
        with (
            tc.tile_pool(name="const", bufs=1) as const,
            tc.tile_pool(name="resident", bufs=1) as res_pool,
            tc.tile_pool(name="xw", bufs=3) as xw_pool,
            tc.tile_pool(name="work", bufs=6) as work,
            tc.tile_pool(name="dram", bufs=1, space="DRAM") as dram,
        ):
            ones_stage = const.tile([P, P], f32)
            nc.vector.memset(ones_stage[:], 1.0)
            ones_col = const.tile([P, 1], md)
            nc.scalar.copy(ones_col[:], ones_stage[:, 0:1])
            ones_row = const.tile([1, P], md)
            nc.scalar.copy(ones_row[:], ones_stage[0:1, :])

            # ---- resident activation buffers ----
            QT = [res_pool.tile([P, T], md, name=f"QT{h}") for h in range(HPC)]
            KT = [res_pool.tile([P, T], md, name=f"KT{h}") for h in range(HPC)]
            V = [res_pool.tile([P, F], md, name=f"V{i}") for i in range(T // P)]

            bounce = dram.tile([T, D], f32, name="bounce")
            rs_out = dram.tile([T // NHG, D], f32, name="rs_out")

            # ---- phase 1: projections (d-outer accumulation) ----
            with tc.tile_pool(name="psum1", bufs=1, space="PSUM") as psum1:
                for tci in range(NTC):
                    xts = []
                    for di in range(ND):
                        xt = xw_pool.tile(
                            [P, TC], md, name=f"xt_{tci}_{di}", tag="xt",
                            bufs=ND + 6,
                        )
                        nc.sync.dma_start(
                            xt[:],
                            xT.ap()[di * P:(di + 1) * P, tci * TC:(tci + 1) * TC],
                        )
                        xts.append(xt)
                    for wname, wT, dest in (("q", wqT, QT), ("k", wkT, KT)):
                        pss = [
                            psum1.tile(
                                [P, TC], f32, name=f"ps_{wname}{h}_{tci}",
                                tag="pq", bufs=4,
                            )
                            for h in range(HPC)
                        ]
                        for di in range(ND):
                            wt = xw_pool.tile(
                                [P, F], md, name=f"w{wname}_{tci}_{di}",
                                tag="wt", bufs=4,
                            )
                            nc.sync.dma_start(
                                wt[:], wT.ap()[di * P:(di + 1) * P, :]
                            )
                            for h in range(HPC):
                                nc.tensor.matmul(
                                    pss[h][:],
                                    wt[:, h * HD:(h + 1) * HD],
                                    xts[di][:],
                                    start=(di == 0),
                                    stop=(di == ND - 1),
                                )
                        for h in range(HPC):
                            nc.scalar.copy(
                                dest[h][:, tci * TC:(tci + 1) * TC], pss[h][:]
                            )
                    pss = [
                        psum1.tile(
                            [P, F], f32, name=f"ps_v{ts}_{tci}", tag="pq", bufs=4
                        )
                        for ts in range(TC // P)
                    ]
                    for di in range(ND):
                        wt = xw_pool.tile(
                            [P, F], md, name=f"wv_{tci}_{di}", tag="wt", bufs=4
                        )
                        nc.sync.dma_start(wt[:], wvT.ap()[di * P:(di + 1) * P, :])
                        for ts in range(TC // P):
                            nc.tensor.matmul(
                                pss[ts][:],
                                xts[di][:, ts * P:(ts + 1) * P],
                                wt[:],
                                start=(di == 0),
                                stop=(di == ND - 1),
                            )
                    for ts in range(TC // P):
                        nc.scalar.copy(V[tci * (TC // P) + ts][:], pss[ts][:])

            # ---- phases 2+3 per q chunk ----
            with tc.tile_pool(name="psum2", bufs=1, space="PSUM") as psum2:
                for qt in range(NTC):
                    outT = {}
                    for h in range(HPC):
                        n_k = (qt + 1) * (TC // P)  # causal: k-subtiles needed
                        ps_out = psum2.tile(
                            [P, TC], f32, name=f"ps_out{qt}_{h}", tag="big",
                            bufs=2,
                        )
                        ps_den = psum2.tile(
                            [1, TC], f32, name=f"ps_den{qt}_{h}", tag="aux",
                            bufs=3,
                        )
                        pts = {}
                        for kt in range(n_k + 1):
                            if kt < n_k:
                                ps_st = psum2.tile(
                                    [P, TC], f32, name=f"ps_st{qt}_{h}_{kt}",
                                    tag="st", bufs=2,
                                )
                                nc.tensor.matmul(
                                    ps_st[:],
                                    KT[h][:, kt * P:(kt + 1) * P],
                                    QT[h][:, qt * TC:(qt + 1) * TC],
                                    start=True,
                                    stop=True,
                                )
                                pt = work.tile(
                                    [P, TC], md, name=f"pt{qt}_{h}_{kt}",
                                    tag="pt", bufs=6,
                                )
                                nc.scalar.activation(
                                    pt[:], ps_st[:],
                                    mybir.ActivationFunctionType.Exp,
                                    scale=SCALE,
                                )
                                dj = kt - qt * (TC // P)
                                if dj >= 0:  # diagonal sub-tile: causal mask
                                    nc.gpsimd.affine_select(
                                        pt[:], pt[:],
                                        pattern=[[1, TC]],
                                        compare_op=mybir.AluOpType.is_ge,
                                        fill=0.0,
                                        base=-128 * dj,
                                        channel_multiplier=-1,
                                    )
                                pts[kt] = pt
                            if kt >= 1:
                                k = kt - 1
                                nc.tensor.matmul(
                                    ps_den[:],
                                    ones_col[:],
                                    pts[k][:],
                                    start=(k == 0),
                                    stop=(k == n_k - 1),
                                )
                                nc.tensor.matmul(
                                    ps_out[:],
                                    V[k][:, h * HD:(h + 1) * HD],
                                    pts[k][:],
                                    start=(k == 0),
                                    stop=(k == n_k - 1),
                                )
                        den = work.tile([1, TC], md, name=f"den{qt}_{h}",
                                        tag="den", bufs=2)
                        nc.vector.reciprocal(den[:], ps_den[:])
                        ps_bc = psum2.tile(
                            [P, TC], f32, name=f"ps_bc{qt}_{h}", tag="aux",
                            bufs=3,
                        )
                        nc.tensor.matmul(
                            ps_bc[:], ones_row[:], den[:],
                            start=True, stop=True,
                        )
                        bc = work.tile([P, TC], f32, name=f"bc{qt}_{h}",
                                       tag="bc", bufs=2)
                        nc.scalar.copy(bc[:], ps_bc[:])
                        ot = work.tile([P, TC], md, name=f"outT{qt}_{h}",
                                       tag="outT", bufs=4)
                        nc.vector.tensor_mul(ot[:], ps_out[:], bc[:])
                        outT[h] = ot

                    # output projection for this q(=t) chunk
                    for et in range(NTC):
                        wos = []
                        for ci in range(HPC):
                            wo = work.tile(
                                [P, TC], md, name=f"wo{qt}_{et}_{ci}",
                                tag="wo", bufs=8,
                            )
                            nc.sync.dma_start(
                                wo[:],
                                woT.ap()[ci * P:(ci + 1) * P,
                                         et * TC:(et + 1) * TC],
                            )
                            wos.append(wo)
                        for ts in range(TC // P):
                            ps_f = psum2.tile(
                                [P, TC], f32, name=f"ps_f{qt}_{ts}_{et}",
                                tag="big", bufs=2,
                            )
                            for ci in range(HPC):
                                nc.tensor.matmul(
                                    ps_f[:],
                                    outT[ci][:, ts * P:(ts + 1) * P],
                                    wos[ci][:],
                                    start=(ci == 0),
                                    stop=(ci == HPC - 1),
                                )
                            fin = work.tile(
                                [P, TC], f32, name=f"fin{qt}_{ts}_{et}",
                                tag="fin", bufs=3,
                            )
                            nc.scalar.copy(fin[:], ps_f[:])
                            row = qt * TC + ts * P
                            nc.sync.dma_start(
                                bounce[row:row + P, et * TC:(et + 1) * TC],
                                fin[:],
                            )

                # ---- phase 4: reduce-scatter over this batch's 4 cores ----
                nc.gpsimd.collective_compute(
                    "ReduceScatter",
                    mybir.AluOpType.add,
                    replica_groups=[[0, 1, 2, 3], [4, 5, 6, 7]],
                    ins=[bounce.opt()],
                    outs=[rs_out.opt()],
                )
                nc.sync.dma_start(out.ap()[:, :], rs_out[:])

    nc.compile()
    return nc


def _get_nc():
    name = os.environ.get("ATTN_MM_DTYPE", "float32r")
    if name not in _CACHE:
        _CACHE[name] = _build(name)
    return _CACHE[name]


last_exec_time_ns = None


def kernel(x, w_qkv, w_out):
    from concourse import bass_utils

    global last_exec_time_ns
    nc = _get_nc()

    x = np.asarray(x, dtype=np.float32)
    w_qkv = np.asarray(w_qkv, dtype=np.float32)
    w_out = np.asarray(w_out, dtype=np.float32)

    xTs = [np.ascontiguousarray(x[b].T) for b in range(B)]
    in_maps = []
    for c in range(N_CORES):
        b, hg = divmod(c, NHG)
        sl = slice(hg * F, (hg + 1) * F)
        in_maps.append({
            "xT": xTs[b],
            "wqT": np.ascontiguousarray(w_qkv[0 * D:1 * D][sl].T),
            "wkT": np.ascontiguousarray(w_qkv[1 * D:2 * D][sl].T),
            "wvT": np.ascontiguousarray(w_qkv[2 * D:3 * D][sl].T),
            "woT": np.ascontiguousarray(w_out[:, sl].T),
        })

    trace = bool(int(os.environ.get("ATTN_TRACE", "0")))
    res = bass_utils.run_bass_kernel_spmd(
        nc, in_maps, core_ids=list(range(N_CORES)), trace=trace
    )
    last_exec_time_ns = res.exec_time_ns

    outs = [res.results[c]["out"] for c in range(N_CORES)]
    return np.stack([
        np.concatenate(outs[:NHG], axis=0),
        np.concatenate(outs[NHG:], axis=0),
    ])


# revision 9
# speedup vs baseline: 1.3529x; 1.3529x over previous
"""Multi-head causal attention (dense transformer block) on 8 Trainium2 cores.

Sharding: 2-way data parallel over batch x 4-way tensor parallel over heads.
Core c handles batch c//4 and heads 4*(c%4) .. 4*(c%4)+3.

Per-core pipeline (all activation layouts chosen so no on-device transposes
are needed; host pre-transposes x and the weight shards once):
  1. QT/KT [hd, t] and V [t, hd] projections from xT [d, t].
  2. Attention per (head, q-chunk) with scores computed transposed
     (S^T [k, q]), exp without max-subtraction (scores are O(1) so exp is
     safe in fp32), causal masking via affine_select on the diagonal tiles,
     softmax denominators via ones-vector matmuls, AV accumulated as
     out^T [hd, q].
  3. Output projection final[t, e] = sum_c out^T[c, t] * woT[c, e] (partial
     sum over this core's heads).
  4. ReduceScatter over the 4 cores sharing a batch; host concatenates the
     row shards.
"""

import os
import sys

sys.path.insert(0, "/opt/trn_rl_repo")

import numpy as np

N_CORES = 8
B = 2
T = 2048          # sequence length
D = 2048          # model dim
P = 128           # partitions
HD = 128          # head dim
NHG = 4           # head-groups (cores per batch)
HPC = 4           # heads per core
F = HPC * HD      # 512 per-core q/k/v feature width
TC = 512          # token chunk (matmul free dim)
NTC = T // TC     # 4 token chunks
ND = D // P       # 16 d-subtiles
SCALE = float(HD) ** -0.5

_CACHE = {}


def _build(mm_dtype_name: str, reps: int = 1, with_rs: bool = True):
    import concourse.bacc as bacc
    import concourse.mybir as mybir
    import concourse.tile as tile

    dt = mybir.dt
    f32 = dt.float32
    md = getattr(dt, mm_dtype_name)  # dtype of every PE-input tile

    nc = bacc.Bacc(
        "TRN2", target_bir_lowering=False, debug=False, num_devices=N_CORES
    )

    xT = nc.dram_tensor("xT", [D, T], md, kind="ExternalInput")
    wqT = nc.dram_tensor("wqT", [D, F], md, kind="ExternalInput")
    wkT = nc.dram_tensor("wkT", [D, F], md, kind="ExternalInput")
    wvT = nc.dram_tensor("wvT", [D, F], md, kind="ExternalInput")
    woT = nc.dram_tensor("woT", [F, D], md, kind="ExternalInput")
    out = nc.dram_tensor("out", [T // NHG, D], f32, kind="ExternalOutput")

    with nc.allow_low_precision(reason="float32r matmul-input tiles"), \
         tile.TileContext(nc) as tc:
        with (
            tc.tile_pool(name="const", bufs=1) as const,
            tc.tile_pool(name="resident", bufs=1) as res_pool,
            tc.tile_pool(name="xw", bufs=3) as xw_pool,
            tc.tile_pool(name="work", bufs=6) as work,
            tc.tile_pool(name="dram", bufs=1, space="DRAM") as dram,
        ):
            ones_stage = const.tile([P, P], f32)
            nc.vector.memset(ones_stage[:], 1.0)
            ones_col = const.tile([P, 1], md)
            nc.scalar.copy(ones_col[:], ones_stage[:, 0:1])
            ones_row = const.tile([1, P], md)
            nc.scalar.copy(ones_row[:], ones_stage[0:1, :])

            # ---- resident activation buffers ----
            QT = [res_pool.tile([P, T], md, name=f"QT{h}") for h in range(HPC)]
            KT = [res_pool.tile([P, T], md, name=f"KT{h}") for h in range(HPC)]
            V = [res_pool.tile([P, F], md, name=f"V{i}") for i in range(T // P)]

            bounce = dram.tile([T, D], f32, name="bounce")
            rs_out = dram.tile([T // NHG, D], f32, name="rs_out")

            for rep in range(reps):
                _build_body(nc, tc, mybir, md, f32, rep, xw_pool, work,
                            xT, wqT, wkT, wvT, woT, out,
                            ones_col, ones_row, QT, KT, V, bounce, rs_out,
                            with_rs)

    nc.compile()
    return nc


def _build_body(nc, tc, mybir, md, f32, rep, xw_pool, work,
                xT, wqT, wkT, wvT, woT, out,
                ones_col, ones_row, QT, KT, V, bounce, rs_out,
                with_rs=True):
    if True:
        if True:
            # ---- phase 1: projections (d-outer accumulation) ----
            with tc.tile_pool(name=f"psum1_{rep}", bufs=1, space="PSUM") as psum1:
                for tci in range(NTC):
                    xts = []
                    for di in range(ND):
                        xt = xw_pool.tile(
                            [P, TC], md, name=f"xt_{rep}_{tci}_{di}", tag="xt",
                            bufs=ND + 6,
                        )
                        nc.sync.dma_start(
                            xt[:],
                            xT.ap()[di * P:(di + 1) * P, tci * TC:(tci + 1) * TC],
                        )
                        xts.append(xt)
                    for wname, wT, dest in (("q", wqT, QT), ("k", wkT, KT)):
                        pss = [
                            psum1.tile(
                                [P, TC], f32, name=f"ps_{wname}{h}_{rep}_{tci}",
                                tag="pq", bufs=4,
                            )
                            for h in range(HPC)
                        ]
                        for di in range(ND):
                            wt = xw_pool.tile(
                                [P, F], md, name=f"w{wname}_{rep}_{tci}_{di}",
                                tag="wt", bufs=4,
                            )
                            nc.sync.dma_start(
                                wt[:], wT.ap()[di * P:(di + 1) * P, :]
                            )
                            for h in range(HPC):
                                nc.tensor.matmul(
                                    pss[h][:],
                                    wt[:, h * HD:(h + 1) * HD],
                                    xts[di][:],
                                    start=(di == 0),
                                    stop=(di == ND - 1),
                                )
                        for h in range(HPC):
                            nc.scalar.copy(
                                dest[h][:, tci * TC:(tci + 1) * TC], pss[h][:]
                            )
                    pss = [
                        psum1.tile(
                            [P, F], f32, name=f"ps_v{ts}_{rep}_{tci}", tag="pq", bufs=4
                        )
                        for ts in range(TC // P)
                    ]
                    for di in range(ND):
                        wt = xw_pool.tile(
                            [P, F], md, name=f"wv_{rep}_{tci}_{di}", tag="wt", bufs=4
                        )
                        nc.sync.dma_start(wt[:], wvT.ap()[di * P:(di + 1) * P, :])
                        for ts in range(TC // P):
                            nc.tensor.matmul(
                                pss[ts][:],
                                xts[di][:, ts * P:(ts + 1) * P],
                                wt[:],
                                start=(di == 0),
                                stop=(di == ND - 1),
                            )
                    for ts in range(TC // P):
                        nc.scalar.copy(V[tci * (TC // P) + ts][:], pss[ts][:])

            # ---- phases 2+3 per q chunk ----
            with tc.tile_pool(name=f"psum2_{rep}", bufs=1, space="PSUM") as psum2:
                for qt in range(NTC):
                    outT = {}
                    for h in range(HPC):
                        n_k = (qt + 1) * (TC // P)  # causal: k-subtiles needed
                        ps_out = psum2.tile(
                            [P, TC], f32, name=f"ps_out{rep}_{qt}_{h}", tag="big",
                            bufs=2,
                        )
                        ps_den = psum2.tile(
                            [1, TC], f32, name=f"ps_den{rep}_{qt}_{h}", tag="aux",
                            bufs=3,
                        )
                        pts = {}
                        for kt in range(n_k + 1):
                            if kt < n_k:
                                ps_st = psum2.tile(
                                    [P, TC], f32, name=f"ps_st{rep}_{qt}_{h}_{kt}",
                                    tag="st", bufs=2,
                                )
                                nc.tensor.matmul(
                                    ps_st[:],
                                    KT[h][:, kt * P:(kt + 1) * P],
                                    QT[h][:, qt * TC:(qt + 1) * TC],
                                    start=True,
                                    stop=True,
                                )
                                pt = work.tile(
                                    [P, TC], md, name=f"pt{rep}_{qt}_{h}_{kt}",
                                    tag="pt", bufs=6,
                                )
                                nc.scalar.activation(
                                    pt[:], ps_st[:],
                                    mybir.ActivationFunctionType.Exp,
                                    scale=SCALE,
                                )
                                dj = kt - qt * (TC // P)
                                if dj >= 0:  # diagonal sub-tile: causal mask
                                    nc.gpsimd.affine_select(
                                        pt[:], pt[:],
                                        pattern=[[1, TC]],
                                        compare_op=mybir.AluOpType.is_ge,
                                        fill=0.0,
                                        base=-128 * dj,
                                        channel_multiplier=-1,
                                    )
                                pts[kt] = pt
                            if kt >= 1:
                                k = kt - 1
                                nc.tensor.matmul(
                                    ps_den[:],
                                    ones_col[:],
                                    pts[k][:],
                                    start=(k == 0),
                                    stop=(k == n_k - 1),
                                )
                                nc.tensor.matmul(
                                    ps_out[:],
                                    V[k][:, h * HD:(h + 1) * HD],
                                    pts[k][:],
                                    start=(k == 0),
                                    stop=(k == n_k - 1),
                                )
                        den = work.tile([1, TC], md, name=f"den{rep}_{qt}_{h}",
                                        tag="den", bufs=2)
                        nc.vector.reciprocal(den[:], ps_den[:])
                        ps_bc = psum2.tile(
                            [P, TC], f32, name=f"ps_bc{rep}_{qt}_{h}", tag="aux",
                            bufs=3,
                        )
                        nc.tensor.matmul(
                            ps_bc[:], ones_row[:], den[:],
                            start=True, stop=True,
                        )
                        bc = work.tile([P, TC], f32, name=f"bc{rep}_{qt}_{h}",
                                       tag="bc", bufs=2)
                        nc.scalar.copy(bc[:], ps_bc[:])
                        ot = work.tile([P, TC], md, name=f"outT{rep}_{qt}_{h}",
                                       tag="outT", bufs=4)
                        nc.vector.tensor_mul(ot[:], ps_out[:], bc[:])
                        outT[h] = ot

                    # output projection for this q(=t) chunk
                    for et in range(NTC):
                        wos = []
                        for ci in range(HPC):
                            wo = work.tile(
                                [P, TC], md, name=f"wo{rep}_{qt}_{et}_{ci}",
                                tag="wo", bufs=8,
                            )
                            nc.sync.dma_start(
                                wo[:],
                                woT.ap()[ci * P:(ci + 1) * P,
                                         et * TC:(et + 1) * TC],
                            )
                            wos.append(wo)
                        for ts in range(TC // P):
                            ps_f = psum2.tile(
                                [P, TC], f32, name=f"ps_f{rep}_{qt}_{ts}_{et}",
                                tag="big", bufs=2,
                            )
                            for ci in range(HPC):
                                nc.tensor.matmul(
                                    ps_f[:],
                                    outT[ci][:, ts * P:(ts + 1) * P],
                                    wos[ci][:],
                                    start=(ci == 0),
                                    stop=(ci == HPC - 1),
                                )
                            fin = work.tile(
                                [P, TC], f32, name=f"fin{rep}_{qt}_{ts}_{et}",
                                tag="fin", bufs=3,
                            )
                            nc.scalar.copy(fin[:], ps_f[:])
                            row = qt * TC + ts * P
                            nc.sync.dma_start(
                                bounce[row:row + P, et * TC:(et + 1) * TC],
                                fin[:],
                            )

                # ---- phase 4: reduce-scatter over this batch's 4 cores ----
                if not with_rs:
                    nc.sync.dma_start(out.ap()[:, :], bounce[0:T // NHG, :])
                    return
                nc.gpsimd.collective_compute(
                    "ReduceScatter",
                    mybir.AluOpType.add,
                    replica_groups=[[0, 1, 2, 3], [4, 5, 6, 7]],
                    ins=[bounce.opt()],
                    outs=[rs_out.opt()],
                )
                nc.sync.dma_start(out.ap()[:, :], rs_out[:])

    nc.compile()
    return nc


def _get_nc():
    name = os.environ.get("ATTN_MM_DTYPE", "float32r")
    reps = int(os.environ.get("ATTN_REPS", "1"))
    key = (name, reps)
    if key not in _CACHE:
        _CACHE[key] = _build(name, reps)
    return _CACHE[key]


last_exec_time_ns = None


def kernel(x, w_qkv, w_out):
    from concourse import bass_utils

    global last_exec_time_ns
    nc = _get_nc()

    x = np.asarray(x, dtype=np.float32)
    w_qkv = np.asarray(w_qkv, dtype=np.float32)
    w_out = np.asarray(w_out, dtype=np.float32)

    xTs = [np.ascontiguousarray(x[b].T) for b in range(B)]
    in_maps = []
    for c in range(N_CORES):
        b, hg = divmod(c, NHG)
        sl = slice(hg * F, (hg + 1) * F)
        in_maps.append({
            "xT": xTs[b],
            "wqT": np.ascontiguousarray(w_qkv[0 * D:1 * D][sl].T),
            "wkT": np.ascontiguousarray(w_qkv[1 * D:2 * D][sl].T),
            "wvT": np.ascontiguousarray(w_qkv[2 * D:3 * D][sl].T),
            "woT": np.ascontiguousarray(w_out[:, sl].T),
        })

    trace = bool(int(os.environ.get("ATTN_TRACE", "0")))
    res = bass_utils.run_bass_kernel_spmd(
        nc, in_maps, core_ids=list(range(N_CORES)), trace=trace
    )
    last_exec_time_ns = res.exec_time_ns

    outs = [res.results[c]["out"] for c in range(N_CORES)]
    return np.stack([
        np.concatenate(outs[:NHG], axis=0),
        np.concatenate(outs[NHG:], axis=0),
    ])


# revision 19
# speedup vs baseline: 170.0673x; 125.7074x over previous
"""Multi-head causal attention (dense transformer block) on 8 Trainium2 cores.

Sharding: 2-way data parallel over batch x 4-way tensor parallel over heads.
Core c handles batch c//4 and heads 4*(c%4) .. 4*(c%4)+3.

Per-core pipeline (all activation layouts chosen so no on-device transposes
are needed; host pre-transposes x and the weight shards once):
  1. QT/KT [hd, t] and V [t, hd] projections from xT [d, t]. Weight tiles
     arrive pre-packed on the host so one DMA carries two d-subtiles
     (halves the HWDGE descriptor load).
  2. Attention per (head, q-chunk) with scores computed transposed
     (S^T [k, q]), exp without max-subtraction (scores are O(1) so exp is
     safe in fp32), causal masking via affine_select on the diagonal tiles,
     softmax denominators via ones-vector matmuls, AV accumulated as
     out^T [hd, q].
  3. Output projection final[t, e] = sum_c out^T[c, t] * woT[c, e] (partial
     sum over this core's heads).
  4. ReduceScatter over the 4 cores sharing a batch; host concatenates the
     row shards.

All matmul inputs are float32r (full-rate fp32 PE mode, ~1.5e-4 relative
error per contraction). PSUM accumulation stays fp32. DMA issue is split
across the SP and ACT sequencers; PSUM evictions run on DVE.
"""

import os
import sys

sys.path.insert(0, "/opt/trn_rl_repo")

import numpy as np

N_CORES = 8
B = 2
T = 2048          # sequence length
D = 2048          # model dim
P = 128           # partitions
HD = 128          # head dim
NHG = 4           # head-groups (cores per batch)
HPC = 4           # heads per core
F = HPC * HD      # 512 per-core q/k/v feature width
TC = 512          # token chunk (matmul free dim)
NTC = T // TC     # 4 token chunks
ND = D // P       # 16 d-subtiles
NJ = ND // 2      # 8 packed weight tiles (2 d-subtiles each)
SCALE = float(HD) ** -0.5

_CACHE = {}


def _build(mm_dtype_name: str, reps: int = 1, with_rs: bool = True):
    import concourse.bacc as bacc
    import concourse.mybir as mybir
    import concourse.tile as tile

    dt = mybir.dt
    f32 = dt.float32
    md = getattr(dt, mm_dtype_name)  # dtype of every PE-input tile

    nc = bacc.Bacc(
        "TRN2", target_bir_lowering=False, debug=False, num_devices=N_CORES
    )

    xT = nc.dram_tensor("xT", [D, T], md, kind="ExternalInput")
    # packed: [j*128+p, sub*512+f] = W^T[(2j+sub)*128+p, f]
    wqP = nc.dram_tensor("wqP", [D // 2, 2 * F], md, kind="ExternalInput")
    wkP = nc.dram_tensor("wkP", [D // 2, 2 * F], md, kind="ExternalInput")
    wvP = nc.dram_tensor("wvP", [D // 2, 2 * F], md, kind="ExternalInput")
    woT = nc.dram_tensor("woT", [F, D], md, kind="ExternalInput")
    out = nc.dram_tensor("out", [T // NHG, D], f32, kind="ExternalOutput")

    with nc.allow_low_precision(reason="float32r matmul-input tiles"), \
         tile.TileContext(nc) as tc:
        with (
            tc.tile_pool(name="const", bufs=1) as const,
            tc.tile_pool(name="resident", bufs=1) as res_pool,
            tc.tile_pool(name="dram", bufs=1, space="DRAM") as dram,
        ):
            ones_stage = const.tile([P, P], f32)
            nc.vector.memset(ones_stage[:], 1.0)
            ones_col = const.tile([P, 1], md)
            nc.scalar.copy(ones_col[:], ones_stage[:, 0:1])
            ones_row = const.tile([1, P], md)
            nc.scalar.copy(ones_row[:], ones_stage[0:1, :])

            # ---- resident activation buffers ----
            QT = [res_pool.tile([P, T], md, name=f"QT{h}") for h in range(HPC)]
            KT = [res_pool.tile([P, T], md, name=f"KT{h}") for h in range(HPC)]
            V = [res_pool.tile([P, F], md, name=f"V{i}") for i in range(T // P)]

            bounce = [dram.tile([TC, D], f32, name=f"bounce{qt}")
                      for qt in range(NTC)]
            rs_out = [dram.tile([TC // NHG, D], f32, name=f"rs_out{qt}")
                      for qt in range(NTC)]

            for rep in range(reps):
                _build_body(nc, tc, mybir, md, f32, rep,
                            xT, wqP, wkP, wvP, woT, out,
                            ones_col, ones_row, QT, KT, V,
                            bounce, rs_out, with_rs)

    nc.compile()
    return nc


def _build_body(nc, tc, mybir, md, f32, rep,
                xT, wqP, wkP, wvP, woT, out,
                ones_col, ones_row, QT, KT, V,
                bounce, rs_out, with_rs=True):
    # ---- phase 1: projections ----
    # Two supersteps of 1024 tokens; each loads the packed q/k/v weights
    # once (24 MB instead of 48 MB of weight traffic per pass over x).
    TG = 2 * TC
    with tc.tile_pool(name=f"psum1_{rep}", bufs=1, space="PSUM") as psum1, \
         tc.tile_pool(name=f"xw_{rep}", bufs=3) as xw_pool:
        for tg in range(T // TG):
            xts = []
            for di in range(ND):
                xt = xw_pool.tile(
                    [P, TG], md, name=f"xt_{rep}_{tg}_{di}", tag="xt",
                    bufs=ND + 2,
                )
                nc.sync.dma_start(
                    xt[:],
                    xT.ap()[di * P:(di + 1) * P, tg * TG:(tg + 1) * TG],
                )
                xts.append(xt)
            wts = {}
            for wname, wP in (("q", wqP), ("k", wkP), ("v", wvP)):
                for j in range(NJ):
                    wt = xw_pool.tile(
                        [P, 2 * F], md, name=f"w{wname}_{rep}_{tg}_{j}",
                        tag="wt", bufs=6,
                    )
                    nc.scalar.dma_start(wt[:], wP.ap()[j * P:(j + 1) * P, :])
                    wts[wname, j] = wt
            for wname, dest in (("q", QT), ("k", KT)):
                pss = [
                    psum1.tile(
                        [P, TC], f32, name=f"ps_{wname}{h}{th}_{rep}_{tg}",
                        tag="pq", bufs=8,
                    )
                    for h in range(HPC) for th in range(2)
                ]
                for j in range(NJ):
                    wt = wts[wname, j]
                    for sub in range(2):
                        di = 2 * j + sub
                        for h in range(HPC):
                            for th in range(2):
                                nc.tensor.matmul(
                                    pss[2 * h + th][:],
                                    wt[:, sub * F + h * HD:
                                       sub * F + (h + 1) * HD],
                                    xts[di][:, th * TC:(th + 1) * TC],
                                    start=(di == 0),
                                    stop=(di == ND - 1),
                                )
                for h in range(HPC):
                    for th in range(2):
                        col = tg * TG + th * TC
                        nc.vector.tensor_copy(
                            dest[h][:, col:col + TC], pss[2 * h + th][:]
                        )
            pss = [
                psum1.tile(
                    [P, F], f32, name=f"ps_v{ts}_{rep}_{tg}", tag="pq", bufs=8
                )
                for ts in range(TG // P)
            ]
            for j in range(NJ):
                wt = wts["v", j]
                for sub in range(2):
                    di = 2 * j + sub
                    for ts in range(TG // P):
                        nc.tensor.matmul(
                            pss[ts][:],
                            xts[di][:, ts * P:(ts + 1) * P],
                            wt[:, sub * F:(sub + 1) * F],
                            start=(di == 0),
                            stop=(di == ND - 1),
                        )
            for ts in range(TG // P):
                nc.vector.tensor_copy(V[tg * (TG // P) + ts][:], pss[ts][:])

    # ---- phases 2+3 per q chunk ----
    with tc.tile_pool(name=f"psum2_{rep}", bufs=1, space="PSUM") as psum2, \
         tc.tile_pool(name=f"work_{rep}", bufs=6) as work:
        WO = []
        for ci in range(HPC):
            row = []
            for etp in range(NTC // 2):
                wo = work.tile([P, 2 * TC], md, name=f"WO{rep}_{ci}_{etp}",
                               tag=f"WO{ci}_{etp}", bufs=1)
                nc.scalar.dma_start(
                    wo[:],
                    woT.ap()[ci * P:(ci + 1) * P,
                             etp * 2 * TC:(etp + 1) * 2 * TC],
                )
                row.append(wo)
            WO.append(row)
        for qt in range(NTC):
            outT = {}
            for h in range(HPC):
                n_k = (qt + 1) * (TC // P)  # causal: k-subtiles needed
                ps_out = psum2.tile(
                    [P, TC], f32, name=f"ps_out{rep}_{qt}_{h}", tag="out",
                    bufs=2,
                )
                ps_den = psum2.tile(
                    [1, TC], f32, name=f"ps_den{rep}_{qt}_{h}", tag="aux",
                    bufs=1,
                )
                # diagonal (masked) k-tiles first so their longer
                # exp+mask chains overlap the off-diagonal stream; skew
                # the consuming matmuls 2 stages behind the producers.
                diag0 = qt * (TC // P)
                korder = list(range(diag0, n_k)) + list(range(diag0))
                SKEW = 2
                pts = {}
                for step in range(n_k + SKEW):
                    if step < n_k:
                        kt = korder[step]
                        ps_st = psum2.tile(
                            [P, TC], f32, name=f"ps_st{rep}_{qt}_{h}_{kt}",
                            tag="st", bufs=3,
                        )
                        nc.tensor.matmul(
                            ps_st[:],
                            KT[h][:, kt * P:(kt + 1) * P],
                            QT[h][:, qt * TC:(qt + 1) * TC],
                            start=True,
                            stop=True,
                        )
                        pt = work.tile(
                            [P, TC], md, name=f"pt{rep}_{qt}_{h}_{kt}",
                            tag="pt", bufs=6,
                        )
                        nc.scalar.activation(
                            pt[:], ps_st[:],
                            mybir.ActivationFunctionType.Exp,
                            scale=SCALE,
                        )
                        dj = kt - diag0
                        if dj >= 0:  # diagonal sub-tile: causal mask
                            nc.gpsimd.affine_select(
                                pt[:], pt[:],
                                pattern=[[1, TC]],
                                compare_op=mybir.AluOpType.is_ge,
                                fill=0.0,
                                base=-128 * dj,
                                channel_multiplier=-1,
                            )
                        pts[kt] = pt
                    if step >= SKEW:
                        idx = step - SKEW
                        k = korder[idx]
                        nc.tensor.matmul(
                            ps_den[:],
                            ones_col[:],
                            pts[k][:],
                            start=(idx == 0),
                            stop=(idx == n_k - 1),
                        )
                        nc.tensor.matmul(
                            ps_out[:],
                            V[k][:, h * HD:(h + 1) * HD],
                            pts[k][:],
                            start=(idx == 0),
                            stop=(idx == n_k - 1),
                        )
                den = work.tile([1, TC], md, name=f"den{rep}_{qt}_{h}",
                                tag="den", bufs=2)
                nc.vector.reciprocal(den[:], ps_den[:])
                ps_bc = psum2.tile(
                    [P, TC], f32, name=f"ps_bc{rep}_{qt}_{h}", tag="aux",
                    bufs=1,
                )
                nc.tensor.matmul(
                    ps_bc[:], ones_row[:], den[:],
                    start=True, stop=True,
                )
                bc = work.tile([P, TC], f32, name=f"bc{rep}_{qt}_{h}",
                               tag="bc", bufs=2)
                nc.vector.tensor_copy(bc[:], ps_bc[:])
                ot = work.tile([P, TC], md, name=f"outT{rep}_{qt}_{h}",
                               tag="outT", bufs=4)
                nc.vector.tensor_mul(ot[:], ps_out[:], bc[:])
                outT[h] = ot

            # output projection for this q(=t) chunk (resident weights)
            for etp in range(NTC // 2):
                for ts in range(TC // P):
                    fin = work.tile(
                        [P, 2 * TC], f32, name=f"fin{rep}_{qt}_{ts}_{etp}",
                        tag="fin", bufs=2,
                    )
                    psf = [
                        psum2.tile(
                            [P, TC], f32,
                            name=f"ps_f{rep}_{qt}_{ts}_{etp}_{ee}",
                            tag="f", bufs=2,
                        )
                        for ee in range(2)
                    ]
                    for ci in range(HPC):
                        for ee in range(2):
                            nc.tensor.matmul(
                                psf[ee][:],
                                outT[ci][:, ts * P:(ts + 1) * P],
                                WO[ci][etp][:, ee * TC:(ee + 1) * TC],
                                start=(ci == 0),
                                stop=(ci == HPC - 1),
                            )
                    for ee in range(2):
                        nc.vector.tensor_copy(
                            fin[:, ee * TC:(ee + 1) * TC], psf[ee][:]
                        )
                    nc.sync.dma_start(
                        bounce[qt][ts * P:(ts + 1) * P,
                                   etp * 2 * TC:(etp + 1) * 2 * TC],
                        fin[:],
                    )
            # ---- phase 4: chunked reduce-scatter, overlapped with the
            # next chunk's compute. Core r of each batch group ends up with
            # rows qt*512 + r*128 .. +128; the host interleaves accordingly.
            if with_rs:
                nc.gpsimd.collective_compute(
                    "ReduceScatter",
                    mybir.AluOpType.add,
                    replica_groups=[[0, 1, 2, 3], [4, 5, 6, 7]],
                    ins=[bounce[qt].opt()],
                    outs=[rs_out[qt].opt()],
                )
                nc.sync.dma_start(
                    out.ap()[qt * (TC // NHG):(qt + 1) * (TC // NHG), :],
                    rs_out[qt][:],
                )
            else:
                nc.sync.dma_start(
                    out.ap()[qt * (TC // NHG):(qt + 1) * (TC // NHG), :],
                    bounce[qt][0:TC // NHG, :],
                )




def _get_nc():
    name = os.environ.get("ATTN_MM_DTYPE", "float32r")
    reps = int(os.environ.get("ATTN_REPS", "1"))
    key = (name, reps)
    if key not in _CACHE:
        _CACHE[key] = _build(name, reps)
    return _CACHE[key]


last_exec_time_ns = None


def _pack_w(wT):
    # [2048, 512] -> [1024, 1024]: packed[j*128+p, sub*512+f] =
    # wT[(2j+sub)*128+p, f]
    return np.ascontiguousarray(
        wT.reshape(NJ, 2, P, F).swapaxes(1, 2).reshape(D // 2, 2 * F)
    )


def make_in_maps(x, w_qkv, w_out):
    x = np.asarray(x, dtype=np.float32)
    w_qkv = np.asarray(w_qkv, dtype=np.float32)
    w_out = np.asarray(w_out, dtype=np.float32)
    xTs = [np.ascontiguousarray(x[b].T) for b in range(B)]
    in_maps = []
    for c in range(N_CORES):
        b, hg = divmod(c, NHG)
        sl = slice(hg * F, (hg + 1) * F)
        in_maps.append({
            "xT": xTs[b],
            "wqP": _pack_w(w_qkv[0 * D:1 * D][sl].T),
            "wkP": _pack_w(w_qkv[1 * D:2 * D][sl].T),
            "wvP": _pack_w(w_qkv[2 * D:3 * D][sl].T),
            "woT": np.ascontiguousarray(w_out[:, sl].T),
        })
    return in_maps


def kernel(x, w_qkv, w_out):
    import time

    from concourse import bass_utils

    global last_exec_time_ns
    nc = _get_nc()
    in_maps = make_in_maps(x, w_qkv, w_out)

    trace = bool(int(os.environ.get("ATTN_TRACE", "0")))
    res = None
    last_err = None
    for attempt in range(3):
        try:
            res = bass_utils.run_bass_kernel_spmd(
                nc, in_maps, core_ids=list(range(N_CORES)), trace=trace
            )
            break
        except Exception as e:  # transient axon mesh desyncs
            last_err = e
            time.sleep(10 * (attempt + 1))
    if res is None:
        raise last_err
    last_exec_time_ns = res.exec_time_ns

    outs = [res.results[c]["out"] for c in range(N_CORES)]
    # chunked RS layout: core r of a batch group holds, for each chunk qt,
    # the summed rows qt*TC + r*(TC//NHG) .. +(TC//NHG).
    RW = TC // NHG
    full = []
    for b in range(B):
        arr = np.stack(outs[b * NHG:(b + 1) * NHG])      # [r, NTC*RW, D]
        arr = arr.reshape(NHG, NTC, RW, D).transpose(1, 0, 2, 3)
        full.append(arr.reshape(T, D))
    return np.stack(full)


# revision 22
# speedup vs baseline: 186.3481x; 1.0957x over previous
"""Multi-head causal attention (dense transformer block) on 8 Trainium2 cores.

Sharding: 2-way data parallel over batch x 4-way tensor parallel over heads.
Core c handles batch c//4 and heads 4*(c%4) .. 4*(c%4)+3.

Per-core pipeline (all activation layouts chosen so no on-device transposes
are needed; host pre-transposes x and the weight shards once):
  1. QT/KT [hd, t] and V [t, hd] projections from xT [d, t]. Weight tiles
     arrive pre-packed on the host so one DMA carries two d-subtiles
     (halves the HWDGE descriptor load).
  2. Attention per (head, q-chunk) with scores computed transposed
     (S^T [k, q]), exp without max-subtraction (scores are O(1) so exp is
     safe in fp32), causal masking via affine_select on the diagonal tiles,
     softmax denominators via ones-vector matmuls, AV accumulated as
     out^T [hd, q].
  3. Output projection final[t, e] = sum_c out^T[c, t] * woT[c, e] (partial
     sum over this core's heads).
  4. ReduceScatter over the 4 cores sharing a batch; host concatenates the
     row shards.

All matmul inputs are float32r (full-rate fp32 PE mode, ~1.5e-4 relative
error per contraction). PSUM accumulation stays fp32. DMA issue is split
across the SP and ACT sequencers; PSUM evictions run on DVE.
"""

import os
import sys

sys.path.insert(0, "/opt/trn_rl_repo")

import numpy as np

N_CORES = 8
B = 2
T = 2048          # sequence length
D = 2048          # model dim
P = 128           # partitions
HD = 128          # head dim
NHG = 4           # head-groups (cores per batch)
HPC = 4           # heads per core
F = HPC * HD      # 512 per-core q/k/v feature width
TC = 512          # token chunk (matmul free dim)
NTC = T // TC     # 4 token chunks
ND = D // P       # 16 d-subtiles
NJ = ND // 2      # 8 packed weight tiles (2 d-subtiles each)
SCALE = float(HD) ** -0.5

_CACHE = {}


def _build(mm_dtype_name: str, reps: int = 1, with_rs: bool = True):
    import concourse.bacc as bacc
    import concourse.mybir as mybir
    import concourse.tile as tile

    dt = mybir.dt
    f32 = dt.float32
    md = getattr(dt, mm_dtype_name)  # dtype of every PE-input tile

    nc = bacc.Bacc(
        "TRN2", target_bir_lowering=False, debug=False, num_devices=N_CORES
    )

    xT = nc.dram_tensor("xT", [D, T], md, kind="ExternalInput")
    # packed: [j*128+p, sub*512+f] = W^T[(2j+sub)*128+p, f]
    wqP = nc.dram_tensor("wqP", [D // 2, 2 * F], md, kind="ExternalInput")
    wkP = nc.dram_tensor("wkP", [D // 2, 2 * F], md, kind="ExternalInput")
    wvP = nc.dram_tensor("wvP", [D // 2, 2 * F], md, kind="ExternalInput")
    woT = nc.dram_tensor("woT", [F, D], md, kind="ExternalInput")
    out = nc.dram_tensor("out", [T // NHG, D], f32, kind="ExternalOutput")

    with nc.allow_low_precision(reason="float32r matmul-input tiles"), \
         tile.TileContext(nc) as tc:
        with (
            tc.tile_pool(name="const", bufs=1) as const,
            tc.tile_pool(name="resident", bufs=1) as res_pool,
            tc.tile_pool(name="dram", bufs=1, space="DRAM") as dram,
        ):
            ones_stage = const.tile([P, P], f32)
            nc.vector.memset(ones_stage[:], 1.0)
            ones_col = const.tile([P, 1], md)
            nc.scalar.copy(ones_col[:], ones_stage[:, 0:1])
            ones_row = const.tile([1, P], md)
            nc.scalar.copy(ones_row[:], ones_stage[0:1, :])

            # ---- resident activation buffers ----
            QT = [res_pool.tile([P, T], md, name=f"QT{h}") for h in range(HPC)]
            KT = [res_pool.tile([P, T], md, name=f"KT{h}") for h in range(HPC)]
            V = [res_pool.tile([P, F], md, name=f"V{i}") for i in range(T // P)]

            bounce = [dram.tile([TC, D], f32, name=f"bounce{qt}")
                      for qt in range(NTC)]
            rs_out = [dram.tile([TC // NHG, D], f32, name=f"rs_out{qt}")
                      for qt in range(NTC)]

            for rep in range(reps):
                _build_body(nc, tc, mybir, md, f32, rep,
                            xT, wqP, wkP, wvP, woT, out,
                            ones_col, ones_row, QT, KT, V,
                            bounce, rs_out, with_rs)

    nc.compile()
    return nc


def _build_body(nc, tc, mybir, md, f32, rep,
                xT, wqP, wkP, wvP, woT, out,
                ones_col, ones_row, QT, KT, V,
                bounce, rs_out, with_rs=True):
    # ---- phase 1: projections ----
    # Two supersteps of 1024 tokens; each loads the packed q/k/v weights
    # once (24 MB instead of 48 MB of weight traffic per pass over x).
    TG = 2 * TC
    with tc.tile_pool(name=f"psum1_{rep}", bufs=1, space="PSUM") as psum1, \
         tc.tile_pool(name=f"xw_{rep}", bufs=3) as xw_pool:
        for tg in range(T // TG):
            xts = []
            for di in range(ND):
                xt = xw_pool.tile(
                    [P, TG], md, name=f"xt_{rep}_{tg}_{di}", tag="xt",
                    bufs=ND + 2,
                )
                nc.sync.dma_start(
                    xt[:],
                    xT.ap()[di * P:(di + 1) * P, tg * TG:(tg + 1) * TG],
                )
                xts.append(xt)
            wts = {}
            for wname, wP in (("q", wqP), ("k", wkP), ("v", wvP)):
                for j in range(NJ):
                    wt = xw_pool.tile(
                        [P, 2 * F], md, name=f"w{wname}_{rep}_{tg}_{j}",
                        tag="wt", bufs=6,
                    )
                    nc.scalar.dma_start(wt[:], wP.ap()[j * P:(j + 1) * P, :])
                    wts[wname, j] = wt
            for wname, dest in (("q", QT), ("k", KT)):
                pss = [
                    psum1.tile(
                        [P, TC], f32, name=f"ps_{wname}{h}{th}_{rep}_{tg}",
                        tag="pq", bufs=8,
                    )
                    for h in range(HPC) for th in range(2)
                ]
                for j in range(NJ):
                    wt = wts[wname, j]
                    for sub in range(2):
                        di = 2 * j + sub
                        for h in range(HPC):
                            for th in range(2):
                                nc.tensor.matmul(
                                    pss[2 * h + th][:],
                                    wt[:, sub * F + h * HD:
                                       sub * F + (h + 1) * HD],
                                    xts[di][:, th * TC:(th + 1) * TC],
                                    start=(di == 0),
                                    stop=(di == ND - 1),
                                )
                for h in range(HPC):
                    for th in range(2):
                        col = tg * TG + th * TC
                        nc.vector.tensor_copy(
                            dest[h][:, col:col + TC], pss[2 * h + th][:]
                        )
            pss = [
                psum1.tile(
                    [P, F], f32, name=f"ps_v{ts}_{rep}_{tg}", tag="pq", bufs=8
                )
                for ts in range(TG // P)
            ]
            for j in range(NJ):
                wt = wts["v", j]
                for sub in range(2):
                    di = 2 * j + sub
                    for ts in range(TG // P):
                        nc.tensor.matmul(
                            pss[ts][:],
                            xts[di][:, ts * P:(ts + 1) * P],
                            wt[:, sub * F:(sub + 1) * F],
                            start=(di == 0),
                            stop=(di == ND - 1),
                        )
            for ts in range(TG // P):
                nc.vector.tensor_copy(V[tg * (TG // P) + ts][:], pss[ts][:])

    # ---- phases 2+3 per q chunk ----
    with tc.tile_pool(name=f"psum2_{rep}", bufs=1, space="PSUM") as psum2, \
         tc.tile_pool(name=f"work_{rep}", bufs=6) as work:
        WO = []
        for ci in range(HPC):
            row = []
            for etp in range(NTC // 2):
                wo = work.tile([P, 2 * TC], md, name=f"WO{rep}_{ci}_{etp}",
                               tag=f"WO{ci}_{etp}", bufs=1)
                nc.scalar.dma_start(
                    wo[:],
                    woT.ap()[ci * P:(ci + 1) * P,
                             etp * 2 * TC:(etp + 1) * 2 * TC],
                )
                row.append(wo)
            WO.append(row)
        for qt in range(NTC):
            outT = {}
            for h in range(HPC):
                n_k = (qt + 1) * (TC // P)  # causal: k-subtiles needed
                ps_out = psum2.tile(
                    [P, TC], f32, name=f"ps_out{rep}_{qt}_{h}", tag="out",
                    bufs=2,
                )
                ps_den = psum2.tile(
                    [1, TC], f32, name=f"ps_den{rep}_{qt}_{h}", tag="aux",
                    bufs=1,
                )
                # diagonal (masked) k-tiles first so their longer
                # exp+mask chains overlap the off-diagonal stream; skew
                # the consuming matmuls 2 stages behind the producers.
                diag0 = qt * (TC // P)
                korder = list(range(diag0, n_k)) + list(range(diag0))
                SKEW = 2
                pts = {}
                for step in range(n_k + SKEW):
                    if step < n_k:
                        kt = korder[step]
                        ps_st = psum2.tile(
                            [P, TC], f32, name=f"ps_st{rep}_{qt}_{h}_{kt}",
                            tag="st", bufs=3,
                        )
                        nc.tensor.matmul(
                            ps_st[:],
                            KT[h][:, kt * P:(kt + 1) * P],
                            QT[h][:, qt * TC:(qt + 1) * TC],
                            start=True,
                            stop=True,
                        )
                        pt = work.tile(
                            [P, TC], md, name=f"pt{rep}_{qt}_{h}_{kt}",
                            tag="pt", bufs=6,
                        )
                        nc.scalar.activation(
                            pt[:], ps_st[:],
                            mybir.ActivationFunctionType.Exp,
                            scale=SCALE,
                        )
                        dj = kt - diag0
                        if dj >= 0:  # diagonal sub-tile: causal mask
                            nc.gpsimd.affine_select(
                                pt[:], pt[:],
                                pattern=[[1, TC]],
                                compare_op=mybir.AluOpType.is_ge,
                                fill=0.0,
                                base=-128 * dj,
                                channel_multiplier=-1,
                            )
                        pts[kt] = pt
                    if step >= SKEW:
                        idx = step - SKEW
                        k = korder[idx]
                        nc.tensor.matmul(
                            ps_den[:],
                            ones_col[:],
                            pts[k][:],
                            start=(idx == 0),
                            stop=(idx == n_k - 1),
                        )
                        nc.tensor.matmul(
                            ps_out[:],
                            V[k][:, h * HD:(h + 1) * HD],
                            pts[k][:],
                            start=(idx == 0),
                            stop=(idx == n_k - 1),
                        )
                den = work.tile([1, TC], md, name=f"den{rep}_{qt}_{h}",
                                tag="den", bufs=2)
                nc.vector.reciprocal(den[:], ps_den[:])
                ps_bc = psum2.tile(
                    [P, TC], f32, name=f"ps_bc{rep}_{qt}_{h}", tag="aux",
                    bufs=1,
                )
                nc.tensor.matmul(
                    ps_bc[:], ones_row[:], den[:],
                    start=True, stop=True,
                )
                bc = work.tile([P, TC], f32, name=f"bc{rep}_{qt}_{h}",
                               tag="bc", bufs=2)
                nc.vector.tensor_copy(bc[:], ps_bc[:])
                ot = work.tile([P, TC], md, name=f"outT{rep}_{qt}_{h}",
                               tag="outT", bufs=4)
                nc.vector.tensor_mul(ot[:], ps_out[:], bc[:])
                outT[h] = ot

            # output projection for this q(=t) chunk (resident weights)
            for etp in range(NTC // 2):
                for ts in range(TC // P):
                    fin = work.tile(
                        [P, 2 * TC], f32, name=f"fin{rep}_{qt}_{ts}_{etp}",
                        tag="fin", bufs=2,
                    )
                    psf = [
                        psum2.tile(
                            [P, TC], f32,
                            name=f"ps_f{rep}_{qt}_{ts}_{etp}_{ee}",
                            tag="f", bufs=2,
                        )
                        for ee in range(2)
                    ]
                    for ci in range(HPC):
                        for ee in range(2):
                            nc.tensor.matmul(
                                psf[ee][:],
                                outT[ci][:, ts * P:(ts + 1) * P],
                                WO[ci][etp][:, ee * TC:(ee + 1) * TC],
                                start=(ci == 0),
                                stop=(ci == HPC - 1),
                            )
                    for ee in range(2):
                        nc.vector.tensor_copy(
                            fin[:, ee * TC:(ee + 1) * TC], psf[ee][:]
                        )
                    nc.sync.dma_start(
                        bounce[qt][ts * P:(ts + 1) * P,
                                   etp * 2 * TC:(etp + 1) * 2 * TC],
                        fin[:],
                    )
            # ---- phase 4: chunked reduce-scatter, overlapped with the
            # next chunk's compute. Core r of each batch group ends up with
            # rows qt*512 + r*128 .. +128; the host interleaves accordingly.
            if with_rs:
                nc.gpsimd.collective_compute(
                    "ReduceScatter",
                    mybir.AluOpType.add,
                    replica_groups=[[0, 1, 2, 3], [4, 5, 6, 7]],
                    ins=[bounce[qt].opt()],
                    outs=[rs_out[qt].opt()],
                )
                nc.sync.dma_start(
                    out.ap()[qt * (TC // NHG):(qt + 1) * (TC // NHG), :],
                    rs_out[qt][:],
                )
            else:
                nc.sync.dma_start(
                    out.ap()[qt * (TC // NHG):(qt + 1) * (TC // NHG), :],
                    bounce[qt][0:TC // NHG, :],
                )




def _get_nc():
    name = os.environ.get("ATTN_MM_DTYPE", "float32r")
    reps = int(os.environ.get("ATTN_REPS", "1"))
    key = (name, reps)
    if key not in _CACHE:
        _CACHE[key] = _build(name, reps)
    return _CACHE[key]


last_exec_time_ns = None


def _pack_w(wT):
    # [2048, 512] -> [1024, 1024]: packed[j*128+p, sub*512+f] =
    # wT[(2j+sub)*128+p, f]
    return np.ascontiguousarray(
        wT.reshape(NJ, 2, P, F).swapaxes(1, 2).reshape(D // 2, 2 * F)
    )


def make_in_maps(x, w_qkv, w_out):
    x = np.asarray(x, dtype=np.float32)
    w_qkv = np.asarray(w_qkv, dtype=np.float32)
    w_out = np.asarray(w_out, dtype=np.float32)
    xTs = [np.ascontiguousarray(x[b].T) for b in range(B)]
    in_maps = []
    for c in range(N_CORES):
        b, hg = divmod(c, NHG)
        sl = slice(hg * F, (hg + 1) * F)
        in_maps.append({
            "xT": xTs[b],
            "wqP": _pack_w(w_qkv[0 * D:1 * D][sl].T),
            "wkP": _pack_w(w_qkv[1 * D:2 * D][sl].T),
            "wvP": _pack_w(w_qkv[2 * D:3 * D][sl].T),
            "woT": np.ascontiguousarray(w_out[:, sl].T),
        })
    return in_maps


def kernel(x, w_qkv, w_out):
    import time

    from concourse import bass_utils

    global last_exec_time_ns
    nc = _get_nc()
    in_maps = make_in_maps(x, w_qkv, w_out)

    trace = bool(int(os.environ.get("ATTN_TRACE", "0")))
    res = None
    last_err = None
    for attempt in range(3):
        try:
            res = bass_utils.run_bass_kernel_spmd(
                nc, in_maps, core_ids=list(range(N_CORES)), trace=trace
            )
            break
        except Exception as e:  # transient axon mesh desyncs
            last_err = e
            time.sleep(10 * (attempt + 1))
    if res is None:
        raise last_err
    last_exec_time_ns = res.exec_time_ns

    outs = [res.results[c]["out"] for c in range(N_CORES)]
    # chunked RS layout: core r of a batch group holds, for each chunk qt,
    # the summed rows qt*TC + r*(TC//NHG) .. +(TC//NHG).
    RW = TC // NHG
    full = []
    for b in range(B):
        arr = np.stack(outs[b * NHG:(b + 1) * NHG])      # [r, NTC*RW, D]
        arr = arr.reshape(NHG, NTC, RW, D).transpose(1, 0, 2, 3)
        full.append(arr.reshape(T, D))
    return np.stack(full)
